# revision 14
# baseline (speedup 1.0000x reference)
"""Trainium2 Bass kernel: sparse attention with lightning indexer + top-256.

Self-contained: shards the full inputs over 8 NeuronCores (sequence-parallel,
row-interleaved queries), runs one SPMD Bass/Tile kernel, gathers the output.

v3 design notes (vs v2 @533us):
  - Hadamard rotations cancel inside qi.ki (orthogonal) -> skipped.
  - Index-path precision: qi/ki projections stay SPLIT bf16x2 3-term
    (ah*bh + ah*bl + al*bh, f32 PSUM) -- selection is extremely sensitive
    to projection error (f32r's 12-bit mantissa alone costs 200+ top-256
    flips, 1.8e-2 end-to-end).  The LOGITS corrections (qih*kil + qil*kih)
    move to ONE fp8 DoubleRow matmul (e4m3 stationary x e5m2 moving, 2
    contraction tiles per pass) -- CPU-validated 1.12e-2 vs jax-f32 ref.
  - Bisection top-k: 20 iterations (was 16), bracket 256: threshold
    resolution 2.4e-4 instead of 6.1e-3 -- cuts selection flips 99 -> 12.
  - Loop B PSUM split into two 8-head tiles so P2's per-head qih/fp8-plane
    extraction pipelines with P2's first matmuls instead of serializing.
  - Attention: per-chunk-group (4 key-chunks) mask multiplies issued right
    after that group's exps -> PV starts without the 20us mask stall;
    gpsimd only gets 2 heads (it is 6x slower than DVE).
  - Causal "suffix trick" kept: per key-chunk kc only the query suffix that
    can causally see it participates (widths 32-aligned).
  - Keys stored in within-chunk permuted order (pos = a*16+u) so post-AG
    gathers are free-dim rearranges; host permutes cmask columns to match.
"""

# ---- walrus compat patches ----------------------------------------
"""Patch TileContext._drain_and_barrier: split the final drain's sem waits
across multiple SP nops (walrus in this image rejects >2 sync waits per
TPB_CTRL instruction)."""
import concourse.tile as tile_mod
from concourse.vector_clock import ScopedClock

MAX_WAITS = 1

def _patched_drain_and_barrier(self, tick_clock, wait_clock):
    nc = self.nc
    import concourse.mybir as mybir
    probe = nc.sync.nop(nofuse=True, hint="tile_tail_waits")
    wait_clock.add_sem_waits(probe.ins, ScopedClock({None: tick_clock.global_clock}))
    si = probe.ins.sync_info
    waits = list(si.on_wait or []) if si is not None else []
    if len(waits) > MAX_WAITS:
        probe.ins.sync_info = mybir.SyncInfo(
            on_wait=waits[:MAX_WAITS], on_update=si.on_update
        )
        for i in range(MAX_WAITS, len(waits), MAX_WAITS):
            extra = nc.sync.nop(nofuse=True, hint="tile_tail_waits")
            extra.ins.sync_info = mybir.SyncInfo(
                on_wait=waits[i : i + MAX_WAITS], on_update=[]
            )
    nc.sync.drain()

    nc.all_engine_barrier()
    assert self.sems is not None
    popped = nc._tile_sem_poison_stack.pop()
    assert popped is self._sem_poison
    nc.clear_and_free_semaphores(list(self.sems.allocated().values()))
    nc.all_engine_barrier()

tile_mod.TileContext._drain_and_barrier = _patched_drain_and_barrier


def _split_multi_waits(nc, max_waits=1):
    """Walrus in this image rejects >1 sync-wait on engine instructions.
    Insert same-engine nops, each carrying one wait, before any offender."""
    import concourse.mybir as mybir

    n_split = 0
    for f in nc.m.functions:
        for bb in f.blocks:
            new_insts = []
            for inst in bb.instructions:
                si = inst.sync_info
                waits = list(si.on_wait) if (si and si.on_wait) else []
                if len(waits) > max_waits and inst.engine is not None:
                    head, keep = waits[:-max_waits], waits[-max_waits:]
                    for i in range(0, len(head), max_waits):
                        nop = mybir.InstNoOp(
                            name=f"{inst.name}-waitsplit-{i}",
                            ins=[], outs=[],
                        )
                        nop.engine = inst.engine
                        nop.sync_info = mybir.SyncInfo(
                            on_wait=head[i : i + max_waits], on_update=[]
                        )
                        nc.register_instruction(nop, overwrite=True)
                        new_insts.append(nop)
                        n_split += 1
                    inst.sync_info = mybir.SyncInfo(on_wait=keep, on_update=si.on_update)
                new_insts.append(inst)
            bb.instructions[:] = new_insts
    return n_split


# ---- kernel builder ----------------------------------------

import numpy as np
import concourse.bass as bass
import concourse.mybir as mybir
from concourse.alu_op_type import AluOpType
from concourse.tile import TileContext

F32 = mybir.dt.float32
BF = mybir.dt.bfloat16
FP8E4 = mybir.dt.float8e4
FP8E5 = mybir.dt.float8e5
AFT = mybir.ActivationFunctionType
DR = mybir.MatmulPerfMode.DoubleRow

S = 2048
D = 2048
SQ = 256          # queries per core
HI = 16           # index heads
HD = 16           # attn heads
DI = 128          # head dim (both)
NEGBIG = -1e30
N_ITERS = 20      # bisection iterations
BR = 256.0        # initial bracket half-width
# fp8 correction scales: (qih*SA0)e4m3 x (kil*SB0)e5m2 + (qil*SA1) x (kih*SB1)
SA0, SB0 = 2.0 ** -3, 2.0 ** 3
SA1, SB1 = 2.0 ** 9, 2.0 ** -9

# causal suffix widths per key chunk (32-aligned).  Query p of half R holds
# global t = 1024*R + 8*p + c ; chunk kc needs queries p >= 16*(kc - 8*R).
def _widths(nk, r):
    ws = []
    for kc in range(nk):
        p0 = max(0, 16 * (kc - 8 * r))
        p0 = 32 * (p0 // 32)
        ws.append(128 - p0)
    return ws

W_R = [_widths(8, 0), _widths(16, 1)]
OFF_R = []
for ws in W_R:
    off, o = [], 0
    for w in ws:
        off.append(o)
        o += w
    OFF_R.append(off)
TOT_R = [sum(W_R[0]), sum(W_R[1])]       # 640, 1664


def build_kernel(stages=5, dbg=()):
    dbg = set(dbg)
    nc = bass.Bass()

    # ---- DRAM parameters (per-core inputs) ----
    xh_d = nc.declare_dram_parameter("xh", [D, SQ], BF, isOutput=False)
    xl_d = nc.declare_dram_parameter("xl", [D, SQ], BF, isOutput=False)
    xch_d = nc.declare_dram_parameter("xch", [D, SQ], BF, isOutput=False)
    xcl_d = nc.declare_dram_parameter("xcl", [D, SQ], BF, isOutput=False)
    wqh_d = nc.declare_dram_parameter("wqh", [D, HI * DI], BF, isOutput=False)
    wql_d = nc.declare_dram_parameter("wql", [D, HI * DI], BF, isOutput=False)
    wkih_d = nc.declare_dram_parameter("wkih", [D, DI], BF, isOutput=False)
    wkil_d = nc.declare_dram_parameter("wkil", [D, DI], BF, isOutput=False)
    wiwh_d = nc.declare_dram_parameter("wiwh", [D, HI], BF, isOutput=False)
    wiwl_d = nc.declare_dram_parameter("wiwl", [D, HI], BF, isOutput=False)
    wqs_d = nc.declare_dram_parameter("wqs", [D, HD * DI], BF, isOutput=False)
    wkb_d = nc.declare_dram_parameter("wkb", [D, DI], BF, isOutput=False)
    wvb_d = nc.declare_dram_parameter("wvb", [D, DI], BF, isOutput=False)
    cmask_d = nc.declare_dram_parameter("cmask", [3, 128, 1024], F32, isOutput=False)
    wob_d = nc.declare_dram_parameter("wob", [HD * DI, D], BF, isOutput=False)
    outs = {}
    if stages >= 5:
        outs["out"] = nc.declare_dram_parameter("out", [SQ, D], F32, isOutput=True)

    # ---- internal DRAM for the single AllGather ----
    # planes: 0 = kih [di,256], 1 = kil, 2 = kt [di,256], 3 = v [256,di]
    ag_in = nc.dram_tensor("ag_in", [4, 128 * SQ], BF)
    ag_out = nc.dram_tensor("ag_out", [8, 4, 128 * SQ], BF, addr_space="Shared")

    def dbg_out(name, shape, dt=F32):
        outs[name] = nc.declare_dram_parameter(name, shape, dt, isOutput=True)
        return outs[name][tuple(slice(0, s) for s in shape)]

    with TileContext(nc) as tc:
        with tc.tile_pool(name="res", bufs=1) as res, \
             tc.tile_pool(name="mid", bufs=1) as mid, \
             tc.tile_pool(name="p4_sb", bufs=2) as sp4:
            # persistent tensors
            xh_sb = res.tile([128, 16, SQ], BF, tag="xh")
            qih_sb = mid.tile([128, HI, SQ], BF, tag="qih")
            # fp8 planes in paired-contiguous layout: [h, R, plane, col] /
            # [jblock, plane, col] so the DoubleRow operands are contiguous
            # [2, N] pairs (strided pairs measured 2.5x slower).
            q8_sb = mid.tile([128, HI, 2, 2, 128], FP8E4, tag="q8")
            kihT = mid.tile([128, S], BF, tag="kihT")
            kilT = mid.tile([128, S], BF, tag="kilT")
            k8T = mid.tile([128, 4, 2, 512], FP8E5, tag="k8T")
            kT_sb = res.tile([128, S], BF, tag="kT")
            v_sb = res.tile([128, 16, DI], BF, tag="v")
            qT_sb = res.tile([128, HD, SQ], BF, tag="qT")
            iw_sb = mid.tile([128, 2, HI], F32, tag="iw")
            I0 = mid.tile([128, 1024], F32, tag="I0")
            I1 = mid.tile([128, 2048], F32, tag="I1")
            mask0 = mid.tile([128, 1024], BF, tag="mask0")
            mask1 = mid.tile([128, 2048], BF, tag="mask1")
            maskT0 = res.tile([128, 1024], BF, tag="maskT0")
            maskT1 = res.tile([128, 2048], BF, tag="maskT1")
            pt_all = res.tile([128, HD, 2048], BF, tag="pt")
            oT_sb = res.tile([128, HD, SQ], BF, tag="oT")
            ones_sb = res.tile([128, 1], BF, tag="ones")
            onesrow_b = res.tile([1, 128], BF, tag="onesrow")
            nc.vector.memset(ones_sb[:], 1.0)
            nc.vector.memset(onesrow_b[:], 1.0)

            # ------- loop A: ki (bf16x2) / kt / v / iw projections -------
            with tc.tile_pool(name="pa_sb", bufs=2) as sp, \
                 tc.tile_pool(name="pa_ps", bufs=1, space="PSUM") as pp:
                ki_ps = pp.tile([128, SQ], F32, tag="ki_ps")
                kt_ps = pp.tile([128, SQ], F32, tag="kt_ps")
                v_ps = pp.tile([128, 2, DI], F32, tag="v_ps")
                iw_ps = pp.tile([128, 2, HI], F32, tag="iw_ps")
                for k in range(16):
                    ksl = slice(k * 128, (k + 1) * 128)
                    xh_k = xh_sb[:, k, :]
                    nc.scalar.dma_start(out=xh_k, in_=xh_d[ksl, :])
                    xl_k = sp.tile([128, SQ], BF, tag="xlA")
                    nc.scalar.dma_start(out=xl_k[:], in_=xl_d[ksl, :])
                    xch_k = sp.tile([128, SQ], BF, tag="xch")
                    nc.scalar.dma_start(out=xch_k[:], in_=xch_d[ksl, :])
                    xcl_k = sp.tile([128, SQ], BF, tag="xcl")
                    nc.scalar.dma_start(out=xcl_k[:], in_=xcl_d[ksl, :])
                    wkih_k = sp.tile([128, DI], BF, tag="wkih")
                    nc.sync.dma_start(out=wkih_k[:], in_=wkih_d[ksl, :])
                    wkil_k = sp.tile([128, DI], BF, tag="wkil")
                    nc.sync.dma_start(out=wkil_k[:], in_=wkil_d[ksl, :])
                    wkb_k = sp.tile([128, DI], BF, tag="wkb")
                    nc.sync.dma_start(out=wkb_k[:], in_=wkb_d[ksl, :])
                    wvb_k = sp.tile([128, DI], BF, tag="wvb")
                    nc.sync.dma_start(out=wvb_k[:], in_=wvb_d[ksl, :])
                    wiwh_k = sp.tile([128, HI], BF, tag="wiwh")
                    nc.sync.dma_start(out=wiwh_k[:], in_=wiwh_d[ksl, :])
                    wiwl_k = sp.tile([128, HI], BF, tag="wiwl")
                    nc.sync.dma_start(out=wiwl_k[:], in_=wiwl_d[ksl, :])
                    st, fin = (k == 0), (k == 15)
                    # ki: 3-term split product into one accumulator
                    nc.tensor.matmul(ki_ps[:], wkih_k[:], xch_k[:], start=st,
                                     stop=False)
                    nc.tensor.matmul(ki_ps[:], wkih_k[:], xcl_k[:], start=False,
                                     stop=False)
                    nc.tensor.matmul(ki_ps[:], wkil_k[:], xch_k[:], start=False,
                                     stop=fin)
                    nc.tensor.matmul(kt_ps[:], wkb_k[:], xch_k[:], start=st, stop=fin)
                    for R in range(2):
                        nc.tensor.matmul(v_ps[:, R, :],
                                         xch_k[:, R * 128:(R + 1) * 128],
                                         wvb_k[:], start=(st and R == 0), stop=fin)
                    for R in range(2):
                        xhR = xh_k[:, R * 128:(R + 1) * 128]
                        xlR = xl_k[:, R * 128:(R + 1) * 128]
                        nc.tensor.matmul(iw_ps[:, R, :], xhR, wiwh_k[:],
                                         start=(st and R == 0), stop=False)
                        nc.tensor.matmul(iw_ps[:, R, :], xlR, wiwh_k[:],
                                         start=False, stop=False)
                        nc.tensor.matmul(iw_ps[:, R, :], xhR, wiwl_k[:],
                                         start=False, stop=fin)
                nc.scalar.copy(out=iw_sb[:], in_=iw_ps[:])
                # splits + shards -> bounce DRAM
                kih_sh = sp.tile([128, SQ], BF, tag="kih_sh")
                nc.scalar.copy(out=kih_sh[:], in_=ki_ps[:])
                kil_sh = sp.tile([128, SQ], BF, tag="kil_sh")
                nc.vector.scalar_tensor_tensor(
                    out=kil_sh[:], in0=kih_sh[:], scalar=-1.0, in1=ki_ps[:],
                    op0=AluOpType.mult, op1=AluOpType.add)
                kt_sh = sp.tile([128, SQ], BF, tag="kt_sh")
                nc.scalar.copy(out=kt_sh[:], in_=kt_ps[:])
                v_sh = sp.tile([128, 2, DI], BF, tag="v_sh")
                nc.scalar.copy(out=v_sh[:], in_=v_ps[:])
                nc.sync.dma_start(
                    out=ag_in[0, :].rearrange("(p j) -> p j", p=128), in_=kih_sh[:])
                nc.sync.dma_start(
                    out=ag_in[1, :].rearrange("(p j) -> p j", p=128), in_=kil_sh[:])
                nc.sync.dma_start(
                    out=ag_in[2, :].rearrange("(p j) -> p j", p=128), in_=kt_sh[:])
                nc.sync.dma_start(
                    out=ag_in[3, :].rearrange("(R p d) -> p R d", R=2, p=128),
                    in_=v_sh[:])
                nc.gpsimd.collective_compute(
                    "AllGather", AluOpType.bypass, replica_groups=[list(range(8))],
                    ins=[ag_in[:, :]], outs=[ag_out[:, :, :]])
                # gathers back (contiguous key shards: global t = a*256 + j)
                for pl, dst in ((0, kihT), (1, kilT), (2, kT_sb)):
                    nc.gpsimd.dma_start(
                        out=dst[:].rearrange("p (a b) -> p a b", a=8),
                        in_=ag_out[:, pl, :].rearrange("a (p b) -> p a b", p=128))
                for vh in range(2):
                    nc.gpsimd.dma_start(
                        out=v_sb[:, vh::2, :],
                        in_=ag_out[:, 3, :].rearrange(
                            "a (h p d) -> p a h d", p=128, h=2)[:, :, vh, :])

            # ---------------- loop B: qi (bf16x2, 3-term) ----------------
            # On the final k-chunk each head's accumulation completes in m
            # order, so its qih/fp8-plane extraction is issued right there and
            # overlaps the remaining heads' matmuls.
            with tc.tile_pool(name="pb_sb", bufs=2) as sp, \
                 tc.tile_pool(name="pb_ps", bufs=1, space="PSUM") as pp:
                qi_ps = pp.tile([128, HI, SQ], F32, tag="qi_ps")
                for k in range(16):
                    ksl = slice(k * 128, (k + 1) * 128)
                    wqh_k = sp.tile([128, HI * DI], BF, tag="wqh")
                    nc.sync.dma_start(out=wqh_k[:], in_=wqh_d[ksl, :])
                    wql_k = sp.tile([128, HI * DI], BF, tag="wql")
                    nc.sync.dma_start(out=wql_k[:], in_=wql_d[ksl, :])
                    xl_k = sp.tile([128, SQ], BF, tag="xlB")
                    nc.scalar.dma_start(out=xl_k[:], in_=xl_d[ksl, :])
                    xh_k = xh_sb[:, k, :]
                    for m in range(16):
                        msl = slice(m * 128, (m + 1) * 128)
                        qp = qi_ps[:, m, :]
                        nc.tensor.matmul(qp, wqh_k[:, msl], xh_k,
                                         start=(k == 0 and m % 2 == 0), stop=False)
                        nc.tensor.matmul(qp, wqh_k[:, msl], xl_k[:],
                                         start=False, stop=False)
                        nc.tensor.matmul(qp, wql_k[:, msl], xh_k,
                                         start=False, stop=(k == 15))
                        if k == 15:
                            nc.scalar.copy(out=qih_sb[:, m, :], in_=qp)
                            nc.scalar.activation(
                                out=q8_sb[:, m, :, 0, :],
                                in_=qp.rearrange("p (r c) -> p r c", r=2),
                                func=AFT.Identity, scale=SA0)
                            qlt = sp.tile([128, SQ], F32, tag="qlt")
                            nc.vector.scalar_tensor_tensor(
                                out=qlt[:], in0=qih_sb[:, m, :], scalar=-1.0,
                                in1=qp, op0=AluOpType.mult, op1=AluOpType.add)
                            nc.scalar.activation(
                                out=q8_sb[:, m, :, 1, :],
                                in_=qlt[:].rearrange("p (r c) -> p r c", r=2),
                                func=AFT.Identity, scale=SA1)

            if "qih" in dbg:
                nc.sync.dma_start(out=dbg_out("d_qih", [128, HI * SQ], BF),
                                  in_=qih_sb[:].rearrange("p a b -> p (a b)"))

            # --------- top-k bisection ---------
            # R0 counts on ScalarE (sign trick, whole row); R1 on DVE
            # (is_ge + accum).
            def topk_R(R, I_R, mask_R, maskT_R, NR, use_scalar):
                lo = res.tile([128, 1], F32, tag=f"lo{R}")
                nc.vector.memset(lo[:], -BR)
                thr = res.tile([128, 1], F32, tag=f"thr{R}")
                acc = res.tile([128, 1], F32, tag=f"acc{R}")
                geb = res.tile([128, 1], F32, tag=f"geb{R}")
                for it in range(N_ITERS):
                    w2 = (2.0 * BR) / (2.0 ** (it + 1))
                    if use_scalar:
                        # nthr = -(lo + w2); sum(sign(I + nthr)) = 2*cnt - NR
                        nc.vector.tensor_scalar(out=thr[:], in0=lo[:],
                                                scalar1=-1.0, scalar2=w2,
                                                op0=AluOpType.mult,
                                                op1=AluOpType.subtract)
                        nc.scalar.activation(out=mask_R[:], in_=I_R[:],
                                             func=AFT.Sign, bias=thr[:],
                                             accum_out=acc[:])
                        nc.vector.tensor_scalar(out=geb[:], in0=acc[:],
                                                scalar1=float(511 - NR),
                                                scalar2=None, op0=AluOpType.is_ge)
                    else:
                        nc.vector.tensor_scalar(out=thr[:], in0=lo[:], scalar1=w2,
                                                scalar2=None, op0=AluOpType.add)
                        nc.vector.tensor_scalar(out=mask_R[:], in0=I_R[:],
                                                scalar1=thr[:], scalar2=0.0,
                                                op0=AluOpType.is_ge,
                                                op1=AluOpType.add,
                                                accum_out=acc[:])
                        nc.vector.tensor_scalar(out=geb[:], in0=acc[:],
                                                scalar1=255.5, scalar2=None,
                                                op0=AluOpType.is_ge)
                    nc.vector.scalar_tensor_tensor(out=lo[:], in0=geb[:], scalar=w2,
                                                   in1=lo[:], op0=AluOpType.mult,
                                                   op1=AluOpType.add)
                nc.vector.tensor_scalar(out=mask_R[:], in0=I_R[:], scalar1=lo[:],
                                        scalar2=None, op0=AluOpType.is_ge)
                # transposed mask tiles (full 128-partition blocks; DMA
                # transpose requires base partition 0)
                for kc in range(NR // 128):
                    nc.sync.dma_start_transpose(
                        maskT_R[:, kc * 128:(kc + 1) * 128],
                        mask_R[:, kc * 128:(kc + 1) * 128])

            # ------- P2: logits (bf16 main + fp8-DR corr) + relu sum -> I ----
            if stages >= 2:
                groups = [(0, 0), (1, 0), (1, 1)]
                with tc.tile_pool(name="p2_sb", bufs=2) as sp2, \
                     tc.tile_pool(name="p2_ps", bufs=2, space="PSUM") as pp2:
                    # fp8 moving planes of k (from AG'd bf16 planes)
                    nc.vector.tensor_scalar(
                        out=k8T[:, :, 0, :],
                        in0=kilT[:].rearrange("p (a b) -> p a b", a=4),
                        scalar1=SB0, scalar2=None, op0=AluOpType.mult)
                    nc.vector.tensor_scalar(
                        out=k8T[:, :, 1, :],
                        in0=kihT[:].rearrange("p (a b) -> p a b", a=4),
                        scalar1=SB1, scalar2=None, op0=AluOpType.mult)
                    for gi, (R, sc) in enumerate(groups):
                        I_R = I0 if R == 0 else I1
                        Isl = I_R[:, sc * 1024:(sc + 1) * 1024]
                        # scratch carved out of pt_all (dead until scores)
                        base = 8 * (gi % 2)
                        scr = pt_all[:, base:base + 4, :].bitcast(F32).rearrange(
                            "p a b -> p (a b)")
                        cm_t = scr[:, 0:1024]
                        Ib = scr[:, 1024:2048]
                        nc.scalar.dma_start(out=cm_t, in_=cmask_d[gi, :, :])
                        tsl = slice(R * 128, (R + 1) * 128)
                        for h in range(HI):
                            L_ps = pp2.tile([128, 2, 512], F32, tag="L")
                            for j in range(2):
                                csl = slice(sc * 1024 + j * 512,
                                            sc * 1024 + (j + 1) * 512)
                                nc.tensor.matmul(L_ps[:, j, :],
                                                 qih_sb[:, h, tsl],
                                                 kihT[:, csl],
                                                 start=True, stop=False)
                                nc.tensor.matmul(L_ps[:, j, :],
                                                 q8_sb[:, h, R, :, :],
                                                 k8T[:, sc * 2 + j, :, :],
                                                 start=False, stop=True,
                                                 perf_mode=DR)
                            relu_t = sp2.tile([128, 1024], F32, tag="relu")
                            nc.scalar.activation(
                                out=relu_t[:],
                                in_=L_ps[:].rearrange("p a b -> p (a b)"),
                                func=AFT.Relu)
                            dst = Isl if h < 8 else Ib
                            if h == 0 or h == 8:
                                nc.vector.tensor_scalar(
                                    out=dst, in0=relu_t[:],
                                    scalar1=iw_sb[:, R, h:h + 1],
                                    scalar2=None, op0=AluOpType.mult)
                            else:
                                nc.vector.scalar_tensor_tensor(
                                    out=dst, in0=relu_t[:],
                                    scalar=iw_sb[:, R, h:h + 1],
                                    in1=dst, op0=AluOpType.mult,
                                    op1=AluOpType.add)
                        nc.gpsimd.tensor_tensor(out=Ib, in0=Ib, in1=cm_t,
                                                op=AluOpType.add)
                        nc.gpsimd.tensor_tensor(out=Isl, in0=Isl, in1=Ib,
                                                op=AluOpType.add)
                        if gi == 0 and stages >= 3:
                            # I0 complete: bisect R0 under the rest of P2
                            # (DVE count path; ScalarE is saturated by relus)
                            with nc.allow_low_precision(reason="bf16 mask"):
                                topk_R(0, I0, mask0, maskT0, 1024, False)
                if "I" in dbg:
                    nc.sync.dma_start(out=dbg_out("d_I0", [128, 1024]), in_=I0[:])
                    nc.sync.dma_start(out=dbg_out("d_I1", [128, 2048]), in_=I1[:])

            # ---------------- loop C: q^T (bf16) ----------------
            with tc.tile_pool(name="pc_sb", bufs=5) as sp, \
                 tc.tile_pool(name="pc_ps", bufs=1, space="PSUM") as pp:
                q_psA = pp.tile([128, 8, SQ], F32, tag="q_psA")
                q_psB = pp.tile([128, 8, SQ], F32, tag="q_psB")
                for k in range(16):
                    wq_k = sp.tile([128, HD * DI], BF, tag="wqs")
                    nc.sync.dma_start(out=wq_k[:], in_=wqs_d[k * 128:(k + 1) * 128, :])
                    xh_k = xh_sb[:, k, :]
                    for m in range(16):
                        qp = q_psA[:, m, :] if m < 8 else q_psB[:, m - 8, :]
                        nc.tensor.matmul(qp, wq_k[:, m * 128:(m + 1) * 128],
                                         xh_k, start=(k == 0 and m % 2 == 0),
                                         stop=(k == 15))
                        if k == 15:
                            # head m complete: extract now so the PSUM pool
                            # drains with (not after) the last matmuls
                            if m % 2 == 0:
                                nc.scalar.copy(out=qT_sb[:, m, :], in_=qp)
                            else:
                                nc.vector.tensor_scalar(
                                    out=qT_sb[:, m, :], in0=qp, scalar1=1.0,
                                    scalar2=None, op0=AluOpType.mult)
            if "qT" in dbg:
                nc.sync.dma_start(out=dbg_out("d_qT", [128, HD * SQ], BF),
                                  in_=qT_sb[:].rearrange("p a b -> p (a b)"))

            # --------- P3/P4: masked attention with R1 bisection overlapped ----
            # Order: R0 scores+exp+masks -> bisect R1 (DVE; overlaps R0's PV
            # on PE) -> R0 PV/den/norm -> R1 scores/.../PV.  Issuing bisect R1
            # first would queue R0's mask multiplies behind 20 DVE bisection
            # passes and stall PV ~15us.
            if stages >= 3:
                with nc.allow_low_precision(reason="bf16 attention path"):
                    if stages >= 4:
                        with tc.tile_pool(name="s_ps", bufs=2, space="PSUM") as pps, \
                             tc.tile_pool(name="o_ps", bufs=1, space="PSUM") as ppo, \
                             tc.tile_pool(name="d_ps", bufs=1, space="PSUM") as ppd, \
                             tc.tile_pool(name="r_ps", bufs=1, space="PSUM") as ppr:

                            def att_scores(R):
                                nk = (R + 1) * 8
                                tq0 = R * 128
                                maskT_R = maskT0 if R == 0 else maskT1
                                WS = W_R[R]
                                cb = [kc * 128 + 128 - WS[kc] for kc in range(nk)]
                                GRP = 4
                                for kc in range(nk):
                                    w = WS[kc]
                                    for half in range(2):
                                        s8 = pps.tile([128, 8, 128], F32, tag="s8")
                                        for hl in range(8):
                                            h = half * 8 + hl
                                            nc.tensor.matmul(
                                                s8[:, hl, :w],
                                                kT_sb[:, kc * 128:(kc + 1) * 128],
                                                qT_sb[:, h, tq0 + 128 - w:tq0 + 128],
                                                start=(hl % 4 == 0), stop=True)
                                        nc.scalar.activation(
                                            out=pt_all[:, half * 8:half * 8 + 8,
                                                       cb[kc]:cb[kc] + w],
                                            in_=s8[:, :, :w], func=AFT.Exp)
                                    if kc % GRP == GRP - 1:
                                        g0 = (kc // GRP) * GRP * 128
                                        g1 = (kc + 1) * 128
                                        for h in range(HD):
                                            eng = nc.gpsimd if h < 2 else nc.vector
                                            eng.tensor_tensor(
                                                out=pt_all[:, h, g0:g1],
                                                in0=pt_all[:, h, g0:g1],
                                                in1=maskT_R[:, g0:g1],
                                                op=AluOpType.mult)

                            def att_pv(R):
                                nk = (R + 1) * 8
                                tq0 = R * 128
                                WS = W_R[R]
                                cb = [kc * 128 + 128 - WS[kc] for kc in range(nk)]
                                for half in range(2):
                                    o16 = ppo.tile([128, 8, 128], F32, tag="o16")
                                    for kc in range(nk):
                                        w = WS[kc]
                                        for hl in range(8):
                                            h = half * 8 + hl
                                            nc.tensor.matmul(
                                                o16[:, hl, 128 - w:128],
                                                v_sb[:, kc, :],
                                                pt_all[:, h, cb[kc]:cb[kc] + w],
                                                start=(kc == 0 and hl % 4 == 0),
                                                stop=(kc == nk - 1))
                                    for hg in range(2):
                                        h0 = half * 8 + hg * 4
                                        den_t = ppd.tile([1, 512], F32, tag="den")
                                        dv = den_t[0:1, :].rearrange(
                                            "p (h q) -> p h q", h=4)
                                        for kc in range(nk):
                                            w = WS[kc]
                                            nc.tensor.matmul(
                                                dv[:, :, 128 - w:128], ones_sb[:],
                                                pt_all[:, h0:h0 + 4,
                                                       cb[kc]:cb[kc] + w],
                                                start=(kc == 0), stop=(kc == nk - 1))
                                        logd = sp4.tile([1, 512], F32, tag="logd")
                                        nc.scalar.activation(out=logd[:],
                                                             in_=den_t[:],
                                                             func=AFT.Ln)
                                        rden = sp4.tile([1, 512], BF, tag="rden")
                                        nc.scalar.activation(out=rden[:],
                                                             in_=logd[:],
                                                             func=AFT.Exp,
                                                             scale=-1.0)
                                        rb_ps = ppr.tile([128, 512], F32, tag="rb")
                                        nc.tensor.matmul(rb_ps[:], onesrow_b[:],
                                                         rden[:], start=True,
                                                         stop=True)
                                        rb_sb = sp4.tile([128, 512], F32, tag="rbs")
                                        nc.scalar.copy(out=rb_sb[:], in_=rb_ps[:])
                                        nc.vector.tensor_tensor(
                                            out=oT_sb[:, h0:h0 + 4,
                                                      tq0:tq0 + 128],
                                            in0=o16[:, hg * 4:hg * 4 + 4, :],
                                            in1=rb_sb[:].rearrange(
                                                "p (h q) -> p h q", h=4),
                                            op=AluOpType.mult)

                            att_scores(0)
                            topk_R(1, I1, mask1, maskT1, 2048, False)
                            att_pv(0)
                            att_scores(1)
                            att_pv(1)
                    else:
                        topk_R(1, I1, mask1, maskT1, 2048, False)
                    if "mask" in dbg:
                        nc.sync.dma_start(out=dbg_out("d_mask0", [128, 1024], BF),
                                          in_=mask0[:])
                        nc.sync.dma_start(out=dbg_out("d_mask1", [128, 2048], BF),
                                          in_=mask1[:])
            if stages >= 4 and "oT" in dbg:
                nc.sync.dma_start(out=dbg_out("d_oT", [128, HD * SQ], BF),
                                  in_=oT_sb[:].rearrange("p a b -> p (a b)"))

            # ------- P5: output projection, flipped to out[t, D] (N=512) -------
            if stages >= 5:
                with tc.tile_pool(name="p5_sb", bufs=6) as sp, \
                     tc.tile_pool(name="p5_ps", bufs=1, space="PSUM") as pp:
                    ops0 = pp.tile([128, 4, 512], F32, tag="out_ps0")
                    ops1 = pp.tile([128, 4, 512], F32, tag="out_ps1")
                    for hc in range(16):
                        wo_k = sp.tile([128, D], BF, tag="wob")
                        nc.sync.dma_start(out=wo_k[:],
                                          in_=wob_d[hc * 128:(hc + 1) * 128, :])
                        for R, ops in ((0, ops0), (1, ops1)):
                            for j in range(4):
                                nc.tensor.matmul(
                                    ops[:, j, :], oT_sb[:, hc, R * 128:(R + 1) * 128],
                                    wo_k[:, j * 512:(j + 1) * 512],
                                    start=(hc == 0), stop=(hc == 15))
                    for R, ops in ((0, ops0), (1, ops1)):
                        for j in range(4):
                            o_sb = sp.tile([128, 512], F32, tag="out_sb")
                            if j % 2 == 0:
                                nc.scalar.copy(out=o_sb[:], in_=ops[:, j, :])
                            else:
                                nc.vector.tensor_scalar(
                                    out=o_sb[:], in0=ops[:, j, :], scalar1=1.0,
                                    scalar2=None, op0=AluOpType.mult)
                            nc.sync.dma_start(
                                out=outs["out"][R * 128:(R + 1) * 128,
                                                j * 512:(j + 1) * 512],
                                in_=o_sb[:])

    _split_multi_waits(nc)
    return nc, outs


# ---------------- numpy-side prep (shared by kernel.py and tests) ----------------

def make_in_maps(x, wq_idx, wk_idx, w_iw, wq, wk, wv, wo):
    import ml_dtypes
    bf16 = ml_dtypes.bfloat16

    def split2(a):
        a = np.asarray(a, np.float32)
        ah = a.astype(bf16)
        al = (a - ah.astype(np.float32)).astype(bf16)
        return np.ascontiguousarray(ah), np.ascontiguousarray(al)

    x2 = np.ascontiguousarray(np.asarray(x, np.float32)[0])        # [S, D]
    xT_ = np.ascontiguousarray(x2.T)                                # [D, S]
    wqh_, wql_ = split2(wq_idx)
    wkih_, wkil_ = split2(wk_idx)
    wiwh_, wiwl_ = split2(w_iw)
    wqs_ = (np.asarray(wq, np.float32) * np.float32(DI ** -0.5)).astype(bf16)
    wkb_ = np.asarray(wk, np.float32).astype(bf16)
    wvb_ = np.asarray(wv, np.float32).astype(bf16)
    wob_ = np.asarray(wo, np.float32).astype(bf16)

    maps = []
    p = np.arange(128)
    for c in range(8):
        xs = np.ascontiguousarray(xT_[:, c::8])                    # [D, 256]
        xh_, xl_ = split2(xs)
        xc = np.ascontiguousarray(xT_[:, c * SQ:(c + 1) * SQ])     # [D, 256]
        xch_, xcl_ = split2(xc)
        cm = np.zeros((3, 128, 1024), np.float32)
        groups = [(0, 0), (1, 0), (1, 1)]
        for gi, (R, sc) in enumerate(groups):
            t_glob = 1024 * R + 8 * p + c                          # [128]
            s_glob = sc * 1024 + np.arange(1024)                   # [1024]
            cm[gi] = np.where(s_glob[None, :] <= t_glob[:, None], 0.0, NEGBIG)
        maps.append({
            "xh": xh_, "xl": xl_, "xch": xch_, "xcl": xcl_,
            "wqh": wqh_, "wql": wql_,
            "wkih": wkih_, "wkil": wkil_,
            "wiwh": wiwh_, "wiwl": wiwl_,
            "wqs": wqs_, "wkb": wkb_, "wvb": wvb_, "wob": wob_,
            "cmask": cm,
        })
    return maps


def assemble_output(results):
    out = np.zeros((1, S, D), np.float32)
    for c in range(8):
        out[0, c::8, :] = results[c]["out"]
    return out


# ---- public entry point ----------------------------------------------------

_CACHE = {}


def kernel(x, wq_idx, wk_idx, w_iw, wq, wk, wv, wo):
    import concourse.bass_utils as _bu
    in_maps = make_in_maps(x, wq_idx, wk_idx, w_iw, wq, wk, wv, wo)
    if "nc" not in _CACHE:
        _CACHE["nc"] = build_kernel(stages=5)[0]
    nc = _CACHE["nc"]
    res = _bu.run_bass_kernel_spmd(nc, in_maps, core_ids=list(range(8)))
    return assemble_output(res.results).astype(np.float32)


# revision 19
# speedup vs baseline: 1.0471x; 1.0471x over previous
"""Trainium2 Bass kernel: sparse attention with lightning indexer + top-256.

Self-contained: shards the full inputs over 8 NeuronCores (sequence-parallel,
row-interleaved queries), runs one SPMD Bass/Tile kernel, gathers the output.

v3 design notes (vs v2 @533us):
  - Hadamard rotations cancel inside qi.ki (orthogonal) -> skipped.
  - Index-path precision: qi/ki projections stay SPLIT bf16x2 3-term
    (ah*bh + ah*bl + al*bh, f32 PSUM) -- selection is extremely sensitive
    to projection error (f32r's 12-bit mantissa alone costs 200+ top-256
    flips, 1.8e-2 end-to-end).  The LOGITS corrections (qih*kil + qil*kih)
    move to ONE fp8 DoubleRow matmul (e4m3 stationary x e5m2 moving, 2
    contraction tiles per pass) -- CPU-validated 1.12e-2 vs jax-f32 ref.
  - Bisection top-k: 20 iterations (was 16), bracket 256: threshold
    resolution 2.4e-4 instead of 6.1e-3 -- cuts selection flips 99 -> 12.
  - Loop B PSUM split into two 8-head tiles so P2's per-head qih/fp8-plane
    extraction pipelines with P2's first matmuls instead of serializing.
  - Attention: per-chunk-group (4 key-chunks) mask multiplies issued right
    after that group's exps -> PV starts without the 20us mask stall;
    gpsimd only gets 2 heads (it is 6x slower than DVE).
  - Causal "suffix trick" kept: per key-chunk kc only the query suffix that
    can causally see it participates (widths 32-aligned).
  - Keys stored in within-chunk permuted order (pos = a*16+u) so post-AG
    gathers are free-dim rearranges; host permutes cmask columns to match.
"""

# ---- walrus compat patches ----------------------------------------
"""Patch TileContext._drain_and_barrier: split the final drain's sem waits
across multiple SP nops (walrus in this image rejects >2 sync waits per
TPB_CTRL instruction)."""
import concourse.tile as tile_mod
from concourse.vector_clock import ScopedClock

MAX_WAITS = 1

def _patched_drain_and_barrier(self, tick_clock, wait_clock):
    nc = self.nc
    import concourse.mybir as mybir
    probe = nc.sync.nop(nofuse=True, hint="tile_tail_waits")
    wait_clock.add_sem_waits(probe.ins, ScopedClock({None: tick_clock.global_clock}))
    si = probe.ins.sync_info
    waits = list(si.on_wait or []) if si is not None else []
    if len(waits) > MAX_WAITS:
        probe.ins.sync_info = mybir.SyncInfo(
            on_wait=waits[:MAX_WAITS], on_update=si.on_update
        )
        for i in range(MAX_WAITS, len(waits), MAX_WAITS):
            extra = nc.sync.nop(nofuse=True, hint="tile_tail_waits")
            extra.ins.sync_info = mybir.SyncInfo(
                on_wait=waits[i : i + MAX_WAITS], on_update=[]
            )
    nc.sync.drain()

    nc.all_engine_barrier()
    assert self.sems is not None
    popped = nc._tile_sem_poison_stack.pop()
    assert popped is self._sem_poison
    nc.clear_and_free_semaphores(list(self.sems.allocated().values()))
    nc.all_engine_barrier()

tile_mod.TileContext._drain_and_barrier = _patched_drain_and_barrier


def _split_multi_waits(nc, max_waits=1):
    """Walrus in this image rejects >1 sync-wait on engine instructions.
    Insert same-engine nops, each carrying one wait, before any offender."""
    import concourse.mybir as mybir

    n_split = 0
    for f in nc.m.functions:
        for bb in f.blocks:
            new_insts = []
            for inst in bb.instructions:
                si = inst.sync_info
                waits = list(si.on_wait) if (si and si.on_wait) else []
                if len(waits) > max_waits and inst.engine is not None:
                    head, keep = waits[:-max_waits], waits[-max_waits:]
                    for i in range(0, len(head), max_waits):
                        nop = mybir.InstNoOp(
                            name=f"{inst.name}-waitsplit-{i}",
                            ins=[], outs=[],
                        )
                        nop.engine = inst.engine
                        nop.sync_info = mybir.SyncInfo(
                            on_wait=head[i : i + max_waits], on_update=[]
                        )
                        nc.register_instruction(nop, overwrite=True)
                        new_insts.append(nop)
                        n_split += 1
                    inst.sync_info = mybir.SyncInfo(on_wait=keep, on_update=si.on_update)
                new_insts.append(inst)
            bb.instructions[:] = new_insts
    return n_split


# ---- kernel builder ----------------------------------------

import numpy as np
import concourse.bass as bass
import concourse.mybir as mybir
from concourse.alu_op_type import AluOpType
from concourse.tile import TileContext

F32 = mybir.dt.float32
BF = mybir.dt.bfloat16
FP8E4 = mybir.dt.float8e4
FP8E5 = mybir.dt.float8e5
AFT = mybir.ActivationFunctionType
DR = mybir.MatmulPerfMode.DoubleRow

S = 2048
D = 2048
SQ = 256          # queries per core
HI = 16           # index heads
HD = 16           # attn heads
DI = 128          # head dim (both)
NEGBIG = -1e30
N_ITERS = 20      # bisection iterations
BR = 256.0        # initial bracket half-width
# fp8 correction scales: (qih*SA0)e4m3 x (kil*SB0)e5m2 + (qil*SA1) x (kih*SB1)
SA0, SB0 = 2.0 ** -3, 2.0 ** 3
SA1, SB1 = 2.0 ** 9, 2.0 ** -9

# causal suffix widths per key chunk (32-aligned).  Query p of half R holds
# global t = 1024*R + 8*p + c ; chunk kc needs queries p >= 16*(kc - 8*R).
def _widths(nk, r):
    ws = []
    for kc in range(nk):
        p0 = max(0, 16 * (kc - 8 * r))
        p0 = 32 * (p0 // 32)
        ws.append(128 - p0)
    return ws

W_R = [_widths(8, 0), _widths(16, 1)]
OFF_R = []
for ws in W_R:
    off, o = [], 0
    for w in ws:
        off.append(o)
        o += w
    OFF_R.append(off)
TOT_R = [sum(W_R[0]), sum(W_R[1])]       # 640, 1664


def build_kernel(stages=5, dbg=()):
    dbg = set(dbg)
    nc = bass.Bass()

    # ---- DRAM parameters (per-core inputs) ----
    xh_d = nc.declare_dram_parameter("xh", [D, SQ], BF, isOutput=False)
    xl_d = nc.declare_dram_parameter("xl", [D, SQ], BF, isOutput=False)
    xch_d = nc.declare_dram_parameter("xch", [D, SQ], BF, isOutput=False)
    xcl_d = nc.declare_dram_parameter("xcl", [D, SQ], BF, isOutput=False)
    wqh_d = nc.declare_dram_parameter("wqh", [D, HI * DI], BF, isOutput=False)
    wql_d = nc.declare_dram_parameter("wql", [D, HI * DI], BF, isOutput=False)
    wkih_d = nc.declare_dram_parameter("wkih", [D, DI], BF, isOutput=False)
    wkil_d = nc.declare_dram_parameter("wkil", [D, DI], BF, isOutput=False)
    wiwh_d = nc.declare_dram_parameter("wiwh", [D, HI], BF, isOutput=False)
    wiwl_d = nc.declare_dram_parameter("wiwl", [D, HI], BF, isOutput=False)
    wqs_d = nc.declare_dram_parameter("wqs", [D, HD * DI], BF, isOutput=False)
    wkb_d = nc.declare_dram_parameter("wkb", [D, DI], BF, isOutput=False)
    wvb_d = nc.declare_dram_parameter("wvb", [D, DI], BF, isOutput=False)
    cmask_d = nc.declare_dram_parameter("cmask", [3, 128, 1024], F32, isOutput=False)
    wob_d = nc.declare_dram_parameter("wob", [HD * DI, D], BF, isOutput=False)
    outs = {}
    if stages >= 5:
        outs["out"] = nc.declare_dram_parameter("out", [SQ, D], F32, isOutput=True)

    # ---- internal DRAM for the single AllGather ----
    # planes: 0 = kih [di,256], 1 = kil, 2 = kt [di,256], 3 = v [256,di]
    ag_in = nc.dram_tensor("ag_in", [4, 128 * SQ], BF)
    ag_out = nc.dram_tensor("ag_out", [8, 4, 128 * SQ], BF, addr_space="Shared")

    def dbg_out(name, shape, dt=F32):
        outs[name] = nc.declare_dram_parameter(name, shape, dt, isOutput=True)
        return outs[name][tuple(slice(0, s) for s in shape)]

    with TileContext(nc) as tc:
        with tc.tile_pool(name="res", bufs=1) as res, \
             tc.tile_pool(name="mid", bufs=1) as mid, \
             tc.tile_pool(name="p4_sb", bufs=2) as sp4:
            # persistent tensors
            xh_sb = res.tile([128, 16, SQ], BF, tag="xh")
            qih_sb = mid.tile([128, HI, SQ], BF, tag="qih")
            qil_sb = mid.tile([128, HI, SQ], BF, tag="qil")
            kihT = mid.tile([128, S], BF, tag="kihT")
            kilT = mid.tile([128, S], BF, tag="kilT")
            kT_sb = res.tile([128, S], BF, tag="kT")
            v_sb = res.tile([128, 16, DI], BF, tag="v")
            qT_sb = res.tile([128, HD, SQ], BF, tag="qT")
            iw_sb = mid.tile([128, 2, HI], F32, tag="iw")
            I0 = mid.tile([128, 1024], F32, tag="I0")
            I1 = mid.tile([128, 2048], F32, tag="I1")
            mask0 = mid.tile([128, 1024], BF, tag="mask0")
            mask1 = mid.tile([128, 2048], BF, tag="mask1")
            maskT0 = res.tile([128, 1024], BF, tag="maskT0")
            maskT1 = res.tile([128, 2048], BF, tag="maskT1")
            pt_all = res.tile([128, HD, 2048], BF, tag="pt")
            oT_sb = res.tile([128, HD, SQ], BF, tag="oT")
            ones_sb = res.tile([128, 1], BF, tag="ones")
            onesrow_b = res.tile([1, 128], BF, tag="onesrow")
            nc.vector.memset(ones_sb[:], 1.0)
            nc.vector.memset(onesrow_b[:], 1.0)

            # ------- loop A: ki (bf16x2) / kt / v / iw projections -------
            with tc.tile_pool(name="pa_sb", bufs=2) as sp, \
                 tc.tile_pool(name="pa_ps", bufs=1, space="PSUM") as pp:
                ki_ps = pp.tile([128, SQ], F32, tag="ki_ps")
                kt_ps = pp.tile([128, SQ], F32, tag="kt_ps")
                v_ps = pp.tile([128, 2, DI], F32, tag="v_ps")
                iw_ps = pp.tile([128, 2, HI], F32, tag="iw_ps")
                for k in range(16):
                    ksl = slice(k * 128, (k + 1) * 128)
                    xh_k = xh_sb[:, k, :]
                    nc.scalar.dma_start(out=xh_k, in_=xh_d[ksl, :])
                    xl_k = sp.tile([128, SQ], BF, tag="xlA")
                    nc.scalar.dma_start(out=xl_k[:], in_=xl_d[ksl, :])
                    xch_k = sp.tile([128, SQ], BF, tag="xch")
                    nc.scalar.dma_start(out=xch_k[:], in_=xch_d[ksl, :])
                    xcl_k = sp.tile([128, SQ], BF, tag="xcl")
                    nc.scalar.dma_start(out=xcl_k[:], in_=xcl_d[ksl, :])
                    wkih_k = sp.tile([128, DI], BF, tag="wkih")
                    nc.sync.dma_start(out=wkih_k[:], in_=wkih_d[ksl, :])
                    wkil_k = sp.tile([128, DI], BF, tag="wkil")
                    nc.sync.dma_start(out=wkil_k[:], in_=wkil_d[ksl, :])
                    wkb_k = sp.tile([128, DI], BF, tag="wkb")
                    nc.sync.dma_start(out=wkb_k[:], in_=wkb_d[ksl, :])
                    wvb_k = sp.tile([128, DI], BF, tag="wvb")
                    nc.sync.dma_start(out=wvb_k[:], in_=wvb_d[ksl, :])
                    wiwh_k = sp.tile([128, HI], BF, tag="wiwh")
                    nc.sync.dma_start(out=wiwh_k[:], in_=wiwh_d[ksl, :])
                    wiwl_k = sp.tile([128, HI], BF, tag="wiwl")
                    nc.sync.dma_start(out=wiwl_k[:], in_=wiwl_d[ksl, :])
                    st, fin = (k == 0), (k == 15)
                    # ki: 3-term split product into one accumulator
                    nc.tensor.matmul(ki_ps[:], wkih_k[:], xch_k[:], start=st,
                                     stop=False)
                    nc.tensor.matmul(ki_ps[:], wkih_k[:], xcl_k[:], start=False,
                                     stop=False)
                    nc.tensor.matmul(ki_ps[:], wkil_k[:], xch_k[:], start=False,
                                     stop=fin)
                    nc.tensor.matmul(kt_ps[:], wkb_k[:], xch_k[:], start=st, stop=fin)
                    for R in range(2):
                        nc.tensor.matmul(v_ps[:, R, :],
                                         xch_k[:, R * 128:(R + 1) * 128],
                                         wvb_k[:], start=(st and R == 0), stop=fin)
                    for R in range(2):
                        xhR = xh_k[:, R * 128:(R + 1) * 128]
                        xlR = xl_k[:, R * 128:(R + 1) * 128]
                        nc.tensor.matmul(iw_ps[:, R, :], xhR, wiwh_k[:],
                                         start=(st and R == 0), stop=False)
                        nc.tensor.matmul(iw_ps[:, R, :], xlR, wiwh_k[:],
                                         start=False, stop=False)
                        nc.tensor.matmul(iw_ps[:, R, :], xhR, wiwl_k[:],
                                         start=False, stop=fin)
                nc.scalar.copy(out=iw_sb[:], in_=iw_ps[:])
                # splits + shards -> bounce DRAM
                kih_sh = sp.tile([128, SQ], BF, tag="kih_sh")
                nc.scalar.copy(out=kih_sh[:], in_=ki_ps[:])
                kil_sh = sp.tile([128, SQ], BF, tag="kil_sh")
                nc.vector.scalar_tensor_tensor(
                    out=kil_sh[:], in0=kih_sh[:], scalar=-1.0, in1=ki_ps[:],
                    op0=AluOpType.mult, op1=AluOpType.add)
                kt_sh = sp.tile([128, SQ], BF, tag="kt_sh")
                nc.scalar.copy(out=kt_sh[:], in_=kt_ps[:])
                v_sh = sp.tile([128, 2, DI], BF, tag="v_sh")
                nc.scalar.copy(out=v_sh[:], in_=v_ps[:])
                nc.sync.dma_start(
                    out=ag_in[0, :].rearrange("(p j) -> p j", p=128), in_=kih_sh[:])
                nc.sync.dma_start(
                    out=ag_in[1, :].rearrange("(p j) -> p j", p=128), in_=kil_sh[:])
                nc.sync.dma_start(
                    out=ag_in[2, :].rearrange("(p j) -> p j", p=128), in_=kt_sh[:])
                nc.sync.dma_start(
                    out=ag_in[3, :].rearrange("(R p d) -> p R d", R=2, p=128),
                    in_=v_sh[:])
                nc.gpsimd.collective_compute(
                    "AllGather", AluOpType.bypass, replica_groups=[list(range(8))],
                    ins=[ag_in[:, :]], outs=[ag_out[:, :, :]])
                # gathers back (contiguous key shards: global t = a*256 + j)
                for pl, dst in ((0, kihT), (1, kilT), (2, kT_sb)):
                    nc.gpsimd.dma_start(
                        out=dst[:].rearrange("p (a b) -> p a b", a=8),
                        in_=ag_out[:, pl, :].rearrange("a (p b) -> p a b", p=128))
                for vh in range(2):
                    nc.gpsimd.dma_start(
                        out=v_sb[:, vh::2, :],
                        in_=ag_out[:, 3, :].rearrange(
                            "a (h p d) -> p a h d", p=128, h=2)[:, :, vh, :])

            # ---------------- loop B: qi (bf16x2, 3-term) ----------------
            # On the final k-chunk each head's accumulation completes in m
            # order, so its qih/fp8-plane extraction is issued right there and
            # overlaps the remaining heads' matmuls.
            with tc.tile_pool(name="pb_sb", bufs=2) as sp, \
                 tc.tile_pool(name="pb_ps", bufs=1, space="PSUM") as pp:
                qi_ps = pp.tile([128, HI, SQ], F32, tag="qi_ps")
                for k in range(16):
                    ksl = slice(k * 128, (k + 1) * 128)
                    wqh_k = sp.tile([128, HI * DI], BF, tag="wqh")
                    nc.sync.dma_start(out=wqh_k[:], in_=wqh_d[ksl, :])
                    wql_k = sp.tile([128, HI * DI], BF, tag="wql")
                    nc.sync.dma_start(out=wql_k[:], in_=wql_d[ksl, :])
                    xl_k = sp.tile([128, SQ], BF, tag="xlB")
                    nc.scalar.dma_start(out=xl_k[:], in_=xl_d[ksl, :])
                    xh_k = xh_sb[:, k, :]
                    for m in range(16):
                        msl = slice(m * 128, (m + 1) * 128)
                        qp = qi_ps[:, m, :]
                        nc.tensor.matmul(qp, wqh_k[:, msl], xh_k,
                                         start=(k == 0 and m % 2 == 0), stop=False)
                        nc.tensor.matmul(qp, wqh_k[:, msl], xl_k[:],
                                         start=False, stop=False)
                        nc.tensor.matmul(qp, wql_k[:, msl], xh_k,
                                         start=False, stop=(k == 15))
                        if k == 15:
                            # head m complete: extract split planes now so
                            # they pipeline under the remaining heads' matmuls
                            nc.scalar.copy(out=qih_sb[:, m, :], in_=qp)
                            nc.vector.scalar_tensor_tensor(
                                out=qil_sb[:, m, :], in0=qih_sb[:, m, :],
                                scalar=-1.0, in1=qp,
                                op0=AluOpType.mult, op1=AluOpType.add)

            if "qih" in dbg:
                nc.sync.dma_start(out=dbg_out("d_qih", [128, HI * SQ], BF),
                                  in_=qih_sb[:].rearrange("p a b -> p (a b)"))

            # --------- top-k bisection ---------
            # R0 counts on ScalarE (sign trick, whole row); R1 on DVE
            # (is_ge + accum).
            def topk_R(R, I_R, mask_R, maskT_R, NR, use_scalar):
                lo = res.tile([128, 1], F32, tag=f"lo{R}")
                nc.vector.memset(lo[:], -BR)
                thr = res.tile([128, 1], F32, tag=f"thr{R}")
                acc = res.tile([128, 1], F32, tag=f"acc{R}")
                geb = res.tile([128, 1], F32, tag=f"geb{R}")
                for it in range(N_ITERS):
                    w2 = (2.0 * BR) / (2.0 ** (it + 1))
                    if use_scalar:
                        # nthr = -(lo + w2); sum(sign(I + nthr)) = 2*cnt - NR
                        nc.vector.tensor_scalar(out=thr[:], in0=lo[:],
                                                scalar1=-1.0, scalar2=w2,
                                                op0=AluOpType.mult,
                                                op1=AluOpType.subtract)
                        nc.scalar.activation(out=mask_R[:], in_=I_R[:],
                                             func=AFT.Sign, bias=thr[:],
                                             accum_out=acc[:])
                        nc.vector.tensor_scalar(out=geb[:], in0=acc[:],
                                                scalar1=float(511 - NR),
                                                scalar2=None, op0=AluOpType.is_ge)
                    else:
                        nc.vector.tensor_scalar(out=thr[:], in0=lo[:], scalar1=w2,
                                                scalar2=None, op0=AluOpType.add)
                        nc.vector.tensor_scalar(out=mask_R[:], in0=I_R[:],
                                                scalar1=thr[:], scalar2=0.0,
                                                op0=AluOpType.is_ge,
                                                op1=AluOpType.add,
                                                accum_out=acc[:])
                        nc.vector.tensor_scalar(out=geb[:], in0=acc[:],
                                                scalar1=255.5, scalar2=None,
                                                op0=AluOpType.is_ge)
                    nc.vector.scalar_tensor_tensor(out=lo[:], in0=geb[:], scalar=w2,
                                                   in1=lo[:], op0=AluOpType.mult,
                                                   op1=AluOpType.add)
                nc.vector.tensor_scalar(out=mask_R[:], in0=I_R[:], scalar1=lo[:],
                                        scalar2=None, op0=AluOpType.is_ge)
                # transposed mask tiles (full 128-partition blocks; DMA
                # transpose requires base partition 0)
                for kc in range(NR // 128):
                    nc.sync.dma_start_transpose(
                        maskT_R[:, kc * 128:(kc + 1) * 128],
                        mask_R[:, kc * 128:(kc + 1) * 128])

            # ---------------- loop C: q^T (bf16) ----------------
            # Runs BEFORE P2: the AllGather's barrier+transfer latency means
            # kihT only lands ~220us in; loop C needs no AG data and exactly
            # fills that window.
            with tc.tile_pool(name="pc_sb", bufs=5) as sp, \
                 tc.tile_pool(name="pc_ps", bufs=1, space="PSUM") as pp:
                q_psA = pp.tile([128, 8, SQ], F32, tag="q_psA")
                q_psB = pp.tile([128, 8, SQ], F32, tag="q_psB")
                for k in range(16):
                    wq_k = sp.tile([128, HD * DI], BF, tag="wqs")
                    nc.sync.dma_start(out=wq_k[:], in_=wqs_d[k * 128:(k + 1) * 128, :])
                    xh_k = xh_sb[:, k, :]
                    for m in range(16):
                        qp = q_psA[:, m, :] if m < 8 else q_psB[:, m - 8, :]
                        nc.tensor.matmul(qp, wq_k[:, m * 128:(m + 1) * 128],
                                         xh_k, start=(k == 0 and m % 2 == 0),
                                         stop=(k == 15))
                        if k == 15:
                            if m % 2 == 0:
                                nc.scalar.copy(out=qT_sb[:, m, :], in_=qp)
                            else:
                                nc.vector.tensor_scalar(
                                    out=qT_sb[:, m, :], in0=qp, scalar1=1.0,
                                    scalar2=None, op0=AluOpType.mult)
            if "qT" in dbg:
                nc.sync.dma_start(out=dbg_out("d_qT", [128, HD * SQ], BF),
                                  in_=qT_sb[:].rearrange("p a b -> p (a b)"))

            # ------- P2: index logits (3-term) + weighted relu sum -> I -------
            if stages >= 2:
                groups = [(0, 0), (1, 0), (1, 1)]
                with tc.tile_pool(name="p2_sb", bufs=2) as sp2, \
                     tc.tile_pool(name="p2_ps", bufs=2, space="PSUM") as pp2:
                    for gi, (R, sc) in enumerate(groups):
                        I_R = I0 if R == 0 else I1
                        Isl = I_R[:, sc * 1024:(sc + 1) * 1024]
                        # scratch carved out of pt_all (dead until scores)
                        base = 8 * (gi % 2)
                        scr = pt_all[:, base:base + 4, :].bitcast(F32).rearrange(
                            "p a b -> p (a b)")
                        cm_t = scr[:, 0:1024]
                        Ib = scr[:, 1024:2048]
                        nc.scalar.dma_start(out=cm_t, in_=cmask_d[gi, :, :])
                        tsl = slice(R * 128, (R + 1) * 128)
                        for h in range(HI):
                            L_ps = pp2.tile([128, 2, 512], F32, tag="L")
                            for j in range(2):
                                csl = slice(sc * 1024 + j * 512,
                                            sc * 1024 + (j + 1) * 512)
                                nc.tensor.matmul(L_ps[:, j, :],
                                                 qih_sb[:, h, tsl],
                                                 kihT[:, csl],
                                                 start=True, stop=False)
                                nc.tensor.matmul(L_ps[:, j, :],
                                                 qih_sb[:, h, tsl],
                                                 kilT[:, csl],
                                                 start=False, stop=False)
                                nc.tensor.matmul(L_ps[:, j, :],
                                                 qil_sb[:, h, tsl],
                                                 kihT[:, csl],
                                                 start=False, stop=True)
                            relu_t = sp2.tile([128, 1024], F32, tag="relu")
                            nc.scalar.activation(
                                out=relu_t[:],
                                in_=L_ps[:].rearrange("p a b -> p (a b)"),
                                func=AFT.Relu)
                            dst = Isl if h < 8 else Ib
                            if h == 0 or h == 8:
                                nc.vector.tensor_scalar(
                                    out=dst, in0=relu_t[:],
                                    scalar1=iw_sb[:, R, h:h + 1],
                                    scalar2=None, op0=AluOpType.mult)
                            else:
                                nc.vector.scalar_tensor_tensor(
                                    out=dst, in0=relu_t[:],
                                    scalar=iw_sb[:, R, h:h + 1],
                                    in1=dst, op0=AluOpType.mult,
                                    op1=AluOpType.add)
                        nc.gpsimd.tensor_tensor(out=Ib, in0=Ib, in1=cm_t,
                                                op=AluOpType.add)
                        nc.gpsimd.tensor_tensor(out=Isl, in0=Isl, in1=Ib,
                                                op=AluOpType.add)
                        if gi == 0 and stages >= 3:
                            # I0 complete: bisect R0 under the rest of P2
                            # (DVE count path; ScalarE is saturated by relus)
                            with nc.allow_low_precision(reason="bf16 mask"):
                                topk_R(0, I0, mask0, maskT0, 1024, False)
                if "I" in dbg:
                    nc.sync.dma_start(out=dbg_out("d_I0", [128, 1024]), in_=I0[:])
                    nc.sync.dma_start(out=dbg_out("d_I1", [128, 2048]), in_=I1[:])

            # --------- P3/P4: masked attention with R1 bisection overlapped ----
            # Order: R0 scores+exp+masks -> bisect R1 (DVE; overlaps R0's PV
            # on PE) -> R0 PV/den/norm -> R1 scores/.../PV.  Issuing bisect R1
            # first would queue R0's mask multiplies behind 20 DVE bisection
            # passes and stall PV ~15us.
            if stages >= 3:
                with nc.allow_low_precision(reason="bf16 attention path"):
                    if stages >= 4:
                        with tc.tile_pool(name="s_ps", bufs=2, space="PSUM") as pps, \
                             tc.tile_pool(name="o_ps", bufs=1, space="PSUM") as ppo, \
                             tc.tile_pool(name="d_ps", bufs=1, space="PSUM") as ppd, \
                             tc.tile_pool(name="r_ps", bufs=1, space="PSUM") as ppr:

                            def att_scores(R):
                                nk = (R + 1) * 8
                                tq0 = R * 128
                                maskT_R = maskT0 if R == 0 else maskT1
                                WS = W_R[R]
                                cb = [kc * 128 + 128 - WS[kc] for kc in range(nk)]
                                GRP = 4
                                for kc in range(nk):
                                    w = WS[kc]
                                    for half in range(2):
                                        s8 = pps.tile([128, 8, 128], F32, tag="s8")
                                        for hl in range(8):
                                            h = half * 8 + hl
                                            nc.tensor.matmul(
                                                s8[:, hl, :w],
                                                kT_sb[:, kc * 128:(kc + 1) * 128],
                                                qT_sb[:, h, tq0 + 128 - w:tq0 + 128],
                                                start=(hl % 4 == 0), stop=True)
                                        nc.scalar.activation(
                                            out=pt_all[:, half * 8:half * 8 + 8,
                                                       cb[kc]:cb[kc] + w],
                                            in_=s8[:, :, :w], func=AFT.Exp)
                                    if kc % GRP == GRP - 1:
                                        g0 = (kc // GRP) * GRP * 128
                                        g1 = (kc + 1) * 128
                                        for h in range(HD):
                                            eng = nc.gpsimd if h < 2 else nc.vector
                                            eng.tensor_tensor(
                                                out=pt_all[:, h, g0:g1],
                                                in0=pt_all[:, h, g0:g1],
                                                in1=maskT_R[:, g0:g1],
                                                op=AluOpType.mult)

                            def att_pv(R):
                                nk = (R + 1) * 8
                                tq0 = R * 128
                                WS = W_R[R]
                                cb = [kc * 128 + 128 - WS[kc] for kc in range(nk)]
                                for half in range(2):
                                    o16 = ppo.tile([128, 8, 128], F32, tag="o16")
                                    for kc in range(nk):
                                        w = WS[kc]
                                        for hl in range(8):
                                            h = half * 8 + hl
                                            nc.tensor.matmul(
                                                o16[:, hl, 128 - w:128],
                                                v_sb[:, kc, :],
                                                pt_all[:, h, cb[kc]:cb[kc] + w],
                                                start=(kc == 0 and hl % 4 == 0),
                                                stop=(kc == nk - 1))
                                    for hg in range(2):
                                        h0 = half * 8 + hg * 4
                                        den_t = ppd.tile([1, 512], F32, tag="den")
                                        dv = den_t[0:1, :].rearrange(
                                            "p (h q) -> p h q", h=4)
                                        for kc in range(nk):
                                            w = WS[kc]
                                            nc.tensor.matmul(
                                                dv[:, :, 128 - w:128], ones_sb[:],
                                                pt_all[:, h0:h0 + 4,
                                                       cb[kc]:cb[kc] + w],
                                                start=(kc == 0), stop=(kc == nk - 1))
                                        logd = sp4.tile([1, 512], F32, tag="logd")
                                        nc.scalar.activation(out=logd[:],
                                                             in_=den_t[:],
                                                             func=AFT.Ln)
                                        rden = sp4.tile([1, 512], BF, tag="rden")
                                        nc.scalar.activation(out=rden[:],
                                                             in_=logd[:],
                                                             func=AFT.Exp,
                                                             scale=-1.0)
                                        rb_ps = ppr.tile([128, 512], F32, tag="rb")
                                        nc.tensor.matmul(rb_ps[:], onesrow_b[:],
                                                         rden[:], start=True,
                                                         stop=True)
                                        rb_sb = sp4.tile([128, 512], F32, tag="rbs")
                                        nc.scalar.copy(out=rb_sb[:], in_=rb_ps[:])
                                        nc.vector.tensor_tensor(
                                            out=oT_sb[:, h0:h0 + 4,
                                                      tq0:tq0 + 128],
                                            in0=o16[:, hg * 4:hg * 4 + 4, :],
                                            in1=rb_sb[:].rearrange(
                                                "p (h q) -> p h q", h=4),
                                            op=AluOpType.mult)

                            att_scores(0)
                            topk_R(1, I1, mask1, maskT1, 2048, False)
                            att_pv(0)
                            att_scores(1)
                            att_pv(1)
                    else:
                        topk_R(1, I1, mask1, maskT1, 2048, False)
                    if "mask" in dbg:
                        nc.sync.dma_start(out=dbg_out("d_mask0", [128, 1024], BF),
                                          in_=mask0[:])
                        nc.sync.dma_start(out=dbg_out("d_mask1", [128, 2048], BF),
                                          in_=mask1[:])
            if stages >= 4 and "oT" in dbg:
                nc.sync.dma_start(out=dbg_out("d_oT", [128, HD * SQ], BF),
                                  in_=oT_sb[:].rearrange("p a b -> p (a b)"))

            # ------- P5: output projection, flipped to out[t, D] (N=512) -------
            if stages >= 5:
                with tc.tile_pool(name="p5_sb", bufs=6) as sp, \
                     tc.tile_pool(name="p5_ps", bufs=1, space="PSUM") as pp:
                    ops0 = pp.tile([128, 4, 512], F32, tag="out_ps0")
                    ops1 = pp.tile([128, 4, 512], F32, tag="out_ps1")
                    for hc in range(16):
                        wo_k = sp.tile([128, D], BF, tag="wob")
                        nc.sync.dma_start(out=wo_k[:],
                                          in_=wob_d[hc * 128:(hc + 1) * 128, :])
                        for R, ops in ((0, ops0), (1, ops1)):
                            for j in range(4):
                                nc.tensor.matmul(
                                    ops[:, j, :], oT_sb[:, hc, R * 128:(R + 1) * 128],
                                    wo_k[:, j * 512:(j + 1) * 512],
                                    start=(hc == 0), stop=(hc == 15))
                    for R, ops in ((0, ops0), (1, ops1)):
                        for j in range(4):
                            o_sb = sp.tile([128, 512], F32, tag="out_sb")
                            if j % 2 == 0:
                                nc.scalar.copy(out=o_sb[:], in_=ops[:, j, :])
                            else:
                                nc.vector.tensor_scalar(
                                    out=o_sb[:], in0=ops[:, j, :], scalar1=1.0,
                                    scalar2=None, op0=AluOpType.mult)
                            nc.sync.dma_start(
                                out=outs["out"][R * 128:(R + 1) * 128,
                                                j * 512:(j + 1) * 512],
                                in_=o_sb[:])

    _split_multi_waits(nc)
    return nc, outs


# ---------------- numpy-side prep (shared by kernel.py and tests) ----------------

def make_in_maps(x, wq_idx, wk_idx, w_iw, wq, wk, wv, wo):
    import ml_dtypes
    bf16 = ml_dtypes.bfloat16

    def split2(a):
        a = np.asarray(a, np.float32)
        ah = a.astype(bf16)
        al = (a - ah.astype(np.float32)).astype(bf16)
        return np.ascontiguousarray(ah), np.ascontiguousarray(al)

    x2 = np.ascontiguousarray(np.asarray(x, np.float32)[0])        # [S, D]
    xT_ = np.ascontiguousarray(x2.T)                                # [D, S]
    wqh_, wql_ = split2(wq_idx)
    wkih_, wkil_ = split2(wk_idx)
    wiwh_, wiwl_ = split2(w_iw)
    wqs_ = (np.asarray(wq, np.float32) * np.float32(DI ** -0.5)).astype(bf16)
    wkb_ = np.asarray(wk, np.float32).astype(bf16)
    wvb_ = np.asarray(wv, np.float32).astype(bf16)
    wob_ = np.asarray(wo, np.float32).astype(bf16)

    maps = []
    p = np.arange(128)
    for c in range(8):
        xs = np.ascontiguousarray(xT_[:, c::8])                    # [D, 256]
        xh_, xl_ = split2(xs)
        xc = np.ascontiguousarray(xT_[:, c * SQ:(c + 1) * SQ])     # [D, 256]
        xch_, xcl_ = split2(xc)
        cm = np.zeros((3, 128, 1024), np.float32)
        groups = [(0, 0), (1, 0), (1, 1)]
        for gi, (R, sc) in enumerate(groups):
            t_glob = 1024 * R + 8 * p + c                          # [128]
            s_glob = sc * 1024 + np.arange(1024)                   # [1024]
            cm[gi] = np.where(s_glob[None, :] <= t_glob[:, None], 0.0, NEGBIG)
        maps.append({
            "xh": xh_, "xl": xl_, "xch": xch_, "xcl": xcl_,
            "wqh": wqh_, "wql": wql_,
            "wkih": wkih_, "wkil": wkil_,
            "wiwh": wiwh_, "wiwl": wiwl_,
            "wqs": wqs_, "wkb": wkb_, "wvb": wvb_, "wob": wob_,
            "cmask": cm,
        })
    return maps


def assemble_output(results):
    out = np.zeros((1, S, D), np.float32)
    for c in range(8):
        out[0, c::8, :] = results[c]["out"]
    return out


# ---- public entry point ----------------------------------------------------

_CACHE = {}


def kernel(x, wq_idx, wk_idx, w_iw, wq, wk, wv, wo):
    import concourse.bass_utils as _bu
    in_maps = make_in_maps(x, wq_idx, wk_idx, w_iw, wq, wk, wv, wo)
    if "nc" not in _CACHE:
        _CACHE["nc"] = build_kernel(stages=5)[0]
    nc = _CACHE["nc"]
    res = _bu.run_bass_kernel_spmd(nc, in_maps, core_ids=list(range(8)))
    return assemble_output(res.results).astype(np.float32)
